# revision 21
# baseline (speedup 1.0000x reference)
"""Trainium2 Bass kernel for ViT-style attention with continuous relative
position bias (nn_Attention_18554258718870).

Sharding: data-parallel over batch B=64 across 8 NeuronCores (8 batches per
core); weights / bias table replicated.

Host side computes the tiny bias-table MLP (961x2 -> 961x12) and the
idx_table gather; the [kv, h, q] bias tensor is shipped to the device and
added into the attention scores on-chip via identity matmuls.

Device-side structure (per batch of 8 per core):
- x loaded with one DMA as [65, 4*768]; PE transposes to xT [768, 260].
- q/k projection in transposed layout (12 tiles of [128, 260] f32r).
- v projection in natural layout, interleaved per head as [v_h | ones_h]
  bf16 so the PV matmul computes the softmax denominator for free.
- scores: per head-pair and kv-chunk, ONE 2-bank PSUM slot holds both
  heads' [kv_chunk, 260] score tiles (cols 0:260 / 512:772); the bias is
  accumulated by a bf16 identity matmul (start=True) before the K=64 f32r
  score matmuls (the two heads sit in disjoint PE row-groups 0/64).
  ONE exp per chunk (Act) covers both heads -> probs bf16 [*, 520].
- PV accumulates [d | denom] x probs into one PSUM bank per head; DVE
  stages the denominator to SBUF (reciprocal_approx_fast cannot read PSUM
  on HW - it crashes the NEFF), takes the fast reciprocal and scales.
- output projection computes the natural [tok, 768] layout directly
  (lhsT = aoT chunk, both 512/256 halves through adjacent matmuls sharing
  the stationary operand); the projection bias is added during PSUM
  evacuation against a 128-row replicated bias tile; DMA out per chunk.

PSUM: 2x 2-bank score slots + 4x 1-bank general slots (PV pairs +
projection pipeline). Engine balance: Act = exps + qk/xT evacuations;
DVE = everything else. Key HW lessons baked in: LDWEIGHTS costs ~40ns per
stationary-operand change (group matmuls sharing lhsT); PSUM bank count
is the binding constraint on attention pipelining; fewer/larger PSUM
evacuations beat many small ones.
"""
import math
import sys
from contextlib import ExitStack

sys.path.insert(0, "/opt/trn_rl_repo")

import numpy as np
import ml_dtypes

import concourse.bass as bass
import concourse.bacc as bacc
import concourse.tile as tile
from concourse import mybir
from concourse.bass_utils import run_bass_kernel_spmd
from concourse.masks import make_identity

F32 = mybir.dt.float32
F32R = mybir.dt.float32r
BF16 = mybir.dt.bfloat16

B, N, DIM, H, D = 64, 260, 768, 12, 64
NCORES = 8
BPC = B // NCORES            # batches per core
KC = DIM // 128              # 6 contraction chunks
QC = [(0, 128), (128, 128), (256, 4)]   # token chunks (offset, size)
NG = 65                      # x-load token-group size (4 groups of 65)


def _build_program(repeat=1):
    nc = bacc.Bacc("TRN2", target_bir_lowering=False, debug=False,
                   num_devices=NCORES)

    x_d = nc.dram_tensor("x", [BPC, N, DIM], BF16, kind="ExternalInput").ap()
    wqkv_d = nc.dram_tensor("wqkv", [DIM, 3 * DIM], BF16, kind="ExternalInput").ap()
    wproj_d = nc.dram_tensor("wproj", [DIM, DIM], F32R, kind="ExternalInput").ap()
    pbrep_d = nc.dram_tensor("pbrep", [128, DIM], F32, kind="ExternalInput").ap()
    biasT_d = nc.dram_tensor("biasT", [N, H, N], BF16, kind="ExternalInput").ap()
    y_d = nc.dram_tensor("y", [BPC, N, DIM], F32, kind="ExternalOutput").ap()

    with tile.TileContext(nc) as tc, ExitStack() as ctx:
        const = ctx.enter_context(tc.tile_pool(name="const", bufs=1))
        p_x = ctx.enter_context(tc.tile_pool(name="x", bufs=2))
        p_xt = ctx.enter_context(tc.tile_pool(name="xt", bufs=8))
        p_qk = ctx.enter_context(tc.tile_pool(name="qk", bufs=16))
        p_v2 = ctx.enter_context(tc.tile_pool(name="v2", bufs=5))
        p_pr = ctx.enter_context(tc.tile_pool(name="pr", bufs=12))
        p_rec = ctx.enter_context(tc.tile_pool(name="rec", bufs=8))
        p_aot = ctx.enter_context(tc.tile_pool(name="aot", bufs=8))
        p_yn = ctx.enter_context(tc.tile_pool(name="yn", bufs=4))
        psum = ctx.enter_context(tc.tile_pool(name="psum", bufs=4, space="PSUM"))
        psum2 = ctx.enter_context(tc.tile_pool(name="psum2", bufs=2, space="PSUM"))

        ident = const.tile([128, 128], F32, tag="ident")
        make_identity(nc, ident)
        ident_b = const.tile([128, 128], BF16, tag="identb")
        make_identity(nc, ident_b)

        wqkv = []
        for kc in range(KC):
            t = const.tile([128, 3 * DIM], BF16, tag=f"wqkv{kc}")
            nc.sync.dma_start(out=t, in_=wqkv_d[128 * kc:128 * (kc + 1), :])
            wqkv.append(t)
        wproj = []
        for kc in range(KC):
            t = const.tile([128, DIM], F32R, tag=f"wproj{kc}")
            nc.sync.dma_start(out=t, in_=wproj_d[128 * kc:128 * (kc + 1), :])
            wproj.append(t)
        biasT = []
        for c, (off, pkv) in enumerate(QC):
            t = const.tile([pkv, H * N], BF16, tag=f"biasT{c}")
            nc.sync.dma_start(
                out=t.rearrange("p (h q) -> p h q", h=H),
                in_=biasT_d[off:off + pkv],
            )
            biasT.append(t)
        pbrep = const.tile([128, DIM], F32, tag="pbrep")
        nc.sync.dma_start(out=pbrep, in_=pbrep_d)
        ones64 = const.tile([128, 64], BF16, tag="ones64")
        nc.vector.memset(ones64, 1.0)

        def _body():
          for b in range(BPC):
              # ---- xT [DIM, N] via DMA XBAR transpose (bf16); the XBAR
              # needs row counts divisible by 16, so 256 + 4-row remainder
              xT = []
              for kc in range(KC):
                  t = p_xt.tile([128, N], BF16, tag="xt")
                  nc.sync.dma_start_transpose(
                      t[:, 0:256], x_d[b][0:256, 128 * kc:128 * (kc + 1)])
                  nc.sync.dma_start_transpose(
                      t[:, 256:260], x_d[b][256:260, 128 * kc:128 * (kc + 1)])
                  xT.append(t)

              # ---- qkT = (wq|wk) projection, transposed layout ----
              # emit q/k tile pairs together so attention pair hp unlocks early
              qkT = [None] * (2 * H * D // 128)
              for mi in range(2 * H * D // 128):   # 12 tiles of 128 rows
                  m = (mi // 2) + 6 * (mi % 2)     # 0,6,1,7,2,8,...
                  ps = psum.tile([128, N], F32, tag="ps")
                  for kc in range(KC):
                      nc.tensor.matmul(
                          ps,
                          wqkv[kc][:, 128 * m:128 * (m + 1)],
                          xT[kc],
                          start=(kc == 0), stop=(kc == KC - 1),
                      )
                  t = p_qk.tile([128, N], F32R, tag="qk")
                  nc.scalar.copy(t, ps)
                  qkT[m] = t

              # ---- v natural [N, DIM] interleaved with ones: v2 [N, 2*DIM] bf16 ----
              v2 = []
              for c, (off, pq) in enumerate(QC):
                  t = p_v2.tile([pq, 2 * DIM], BF16, tag="v2")
                  halves = ((0, 512), (512, 256))
                  pss = [psum.tile([pq, nsz], F32, tag="ps", name=f"v2h{i}")
                         for i, (noff, nsz) in enumerate(halves)]
                  for kc in range(KC):
                      # consecutive matmuls share the stationary operand
                      for (noff, nsz), ps in zip(halves, pss):
                          nc.tensor.matmul(
                              ps,
                              xT[kc][:, off:off + pq],
                              wqkv[kc][:, 2 * DIM + noff:2 * DIM + noff + nsz],
                              start=(kc == 0), stop=(kc == KC - 1),
                          )
                  for (noff, nsz), ps in zip(halves, pss):
                      # scatter head blocks of 64 into interleaved [v_h | ones_h]
                      nh = nsz // 64
                      dst = bass.AP(tensor=t.tensor, offset=t.offset + 2 * noff,
                                    ap=[t.ap[0], [128, nh], [1, 64]])
                      nc.vector.tensor_copy(dst, ps.rearrange("p (h d) -> p h d", d=64))
                  ones_dst = bass.AP(tensor=t.tensor, offset=t.offset + 64,
                                     ap=[t.ap[0], [128, H], [1, 64]])
                  ones_src = bass.AP(tensor=ones64.tensor, offset=ones64.offset,
                                     ap=[[ones64.ap[0][0], pq], [0, H], [1, 64]])
                  nc.vector.tensor_copy(ones_dst, ones_src)
                  v2.append(t)

              # ---- attention per head-pair ----
              aoT = [p_aot.tile([128, N], F32R, tag="aot", name=f"aot{i}")
                     for i in range(KC)]
              for hp in range(H // 2):
                  h0, h1 = 2 * hp, 2 * hp + 1
                  qtile = qkT[hp]
                  ktile = qkT[H // 2 + hp]
                  # one 2-bank slot per kv-chunk holding BOTH heads' scores
                  # (h0 at cols 0:260 in bank 0, h1 at cols 512:772 in bank
                  # 1); one exp per chunk covers both heads.
                  pts = []
                  for c, (off, pkv) in enumerate(QC):
                      sc = psum2.tile([pkv, 1024], F32, tag="sc")
                      ib = ident_b[:pkv, :pkv]
                      nc.tensor.matmul(sc[:, 0:N], ib,
                                       biasT[c][:, h0 * N:(h0 + 1) * N],
                                       start=True, stop=False)
                      nc.tensor.matmul(sc[:, 512:512 + N], ib,
                                       biasT[c][:, h1 * N:(h1 + 1) * N],
                                       start=True, stop=False)
                      # the two heads' K=64 score matmuls sit in disjoint PE
                      # row-groups (0 / 64) and can overlap in the array
                      nc.tensor.matmul(sc[:, 0:N], ktile[0:64, off:off + pkv],
                                       qtile[0:64, :], start=False, stop=True)
                      nc.tensor.matmul(sc[:, 512:512 + N],
                                       ktile[64:128, off:off + pkv],
                                       qtile[64:128, :], start=False, stop=True)
                      pt = p_pr.tile([pkv, 2 * N], BF16, tag="pr")
                      nc.scalar.activation(
                          pt, sc.rearrange("p (b q) -> p b q", b=2)[:, :, 0:N],
                          mybir.ActivationFunctionType.Exp)
                      pts.append(pt)

                  pvs = {h: psum.tile([128, N], F32, tag="ps", name=f"pv{h % 2}")
                         for h in (h0, h1)}
                  for c, (off, pkv) in enumerate(QC):
                      for i, h in enumerate((h0, h1)):
                          rhs = pts[c][:, i * N:(i + 1) * N]
                          nc.tensor.matmul(pvs[h], v2[c][:, 128 * h:128 * (h + 1)],
                                           rhs, start=(c == 0), stop=(c == 2))
                  for h in (h0, h1):
                      pv = pvs[h]
                      rec = p_rec.tile([64, N], F32, tag="rec")
                      ssb = p_rec.tile([64, N], F32, tag="ssb")
                      nc.vector.tensor_copy(ssb, pv[64:128, :])
                      nc.vector.reciprocal_approx_fast(out=rec, in_=ssb)
                      nc.vector.tensor_tensor(
                          aoT[h // 2][64 * (h % 2):64 * (h % 2) + 64, :],
                          pv[0:64, :], rec, op=mybir.AluOpType.mult,
                      )

              # ---- output projection, natural layout [tok, DIM] ----
              for c, (off, pq) in enumerate(QC):
                  yn = p_yn.tile([pq, DIM], F32, tag="yn")
                  halves = ((0, 512), (512, 256))
                  pss = [psum.tile([pq, nsz], F32, tag="ps", name=f"pj{i}")
                         for i, (noff, nsz) in enumerate(halves)]
                  for kc in range(KC):
                      for (noff, nsz), ps in zip(halves, pss):
                          nc.tensor.matmul(
                              ps,
                              aoT[kc][:, off:off + pq],
                              wproj[kc][:, noff:noff + nsz],
                              start=(kc == 0), stop=(kc == KC - 1),
                          )
                  for (noff, nsz), ps in zip(halves, pss):
                      nc.vector.tensor_tensor(
                          yn[:, noff:noff + nsz], ps,
                          pbrep[0:pq, noff:noff + nsz], op=mybir.AluOpType.add,
                      )
                  nc.sync.dma_start(out=y_d[b, off:off + pq, :], in_=yn)

        if repeat == 1:
            _body()
        else:
            with tc.For_i(0, repeat, 1):
                _body()

    nc.compile()
    return nc


_PROGRAM = None


def _get_program():
    global _PROGRAM
    if _PROGRAM is None:
        _PROGRAM = _build_program()
    return _PROGRAM


def _host_prep(x, qkv_w, proj_w, proj_b, mlp_w1, mlp_b1, mlp_w2, rel_table,
               idx_table, r_cutoff):
    """Host-side: bias table MLP + gather; weight layout prep."""
    x = np.asarray(x, np.float32)
    qkv_w = np.asarray(qkv_w, np.float32)
    proj_w = np.asarray(proj_w, np.float32)
    proj_b = np.asarray(proj_b, np.float32)

    # continuous position bias table: exact GELU MLP
    hdn = np.asarray(rel_table, np.float64) @ np.asarray(mlp_w1, np.float64).T \
        + np.asarray(mlp_b1, np.float64)
    from numpy import vectorize
    erf = vectorize(math.erf)
    hdn = 0.5 * hdn * (1.0 + erf(hdn / math.sqrt(2.0)))
    bt = (hdn @ np.asarray(mlp_w2, np.float64).T).astype(np.float32)  # [T, H]

    idx = np.asarray(idx_table, np.int64)
    rc = int(np.asarray(r_cutoff))
    tok = np.arange(N)
    has_bias = (tok[:, None] >= rc) & (tok[None, :] >= rc)          # [q, kv]
    bias = np.where(has_bias[:, :, None], bt[idx], 0.0)             # [q, kv, H]
    biasT = np.ascontiguousarray(bias.transpose(1, 2, 0))           # [kv, H, q]
    biasT = biasT.astype(ml_dtypes.bfloat16)

    wqkvT = np.ascontiguousarray(qkv_w.T)                           # [DIM, 3*DIM]
    wqkvT = wqkvT.copy()
    wqkvT[:, :DIM] *= np.float32(0.125)                             # fold 1/sqrt(D)
    wprojT = np.ascontiguousarray(proj_w.T)                         # [DIM, DIM]
    pbrep = np.ascontiguousarray(np.tile(proj_b[None, :], (128, 1)))  # [128, DIM]

    x = x.astype(ml_dtypes.bfloat16)
    wqkvT = wqkvT.astype(ml_dtypes.bfloat16)
    return x, wqkvT, wprojT, pbrep, biasT


def kernel(**inputs):
    x, wqkvT, wprojT, pbrep, biasT = _host_prep(**inputs)
    nc = _get_program()
    in_maps = []
    for c in range(NCORES):
        in_maps.append({
            "x": np.ascontiguousarray(x[c * BPC:(c + 1) * BPC]),
            "wqkv": wqkvT,
            "wproj": wprojT,
            "pbrep": pbrep,
            "biasT": biasT,
        })
    last_err = None
    for attempt in range(3):
        try:
            res = run_bass_kernel_spmd(nc, in_maps, list(range(NCORES)))
            break
        except Exception as e:   # rare transient NRT/axon execution failures
            last_err = e
            import time as _time
            _time.sleep(2.0)
    else:
        raise last_err
    y = np.concatenate([res.results[c]["y"] for c in range(NCORES)], axis=0)
    return y.astype(np.float32)
